# revision 11
# baseline (speedup 1.0000x reference)
"""Trainium2 Bass kernel: attention-LSTM decoder (teacher-forced, 26 steps).

Full inputs -> data-parallel over batch across 8 NeuronCores -> full output.

Per-core layout strategy (B_shard = 64):
  - H_proj^T   [h(4x128), (t,b)]  bf16, precomputed on PE once
  - per step:  pT = W_h2h @ h^T (PE)  ->  arg = Hp^T + pT (DVE bcast add, bf16)
               -> tanh (ACT) -> score dot vs w (PE, 4 PSUM col-groups)
               -> exp (ACT, 4 lanes) -> DMA reshape -> block-diag alpha
               -> context matmul w/ ones-column for softmax denom (PE)
               -> ctx scale 1/S (DVE) -> transpose (PE) -> LSTM gates (PE)
               -> pointwise LSTM with sigmoid(x)=0.5*tanh(x/2)+0.5 (ACT+DVE)
               -> h^T transpose (PE) -> generator matmul (PE) -> DMA out
  All transcendentals use the exp_and_others ACT table set (tanh+exp): a
  single table load for the whole kernel.
"""

import numpy as np
import ml_dtypes

B, T, IN = 512, 64, 512
HID = 512
NCLS = 96
S = 26
NCORES = 8
BS = B // NCORES           # 64 batch rows per core
G4 = 4 * HID               # 2048 LSTM gate width
HFW = IN + 4               # Hflat chunk width: 512 H cols + ones col + 3 pad
BF = ml_dtypes.bfloat16

_CACHE = {}


def _build_nc():
    from contextlib import ExitStack

    import concourse.masks as masks
    import concourse.tile as tile
    from concourse import bacc, mybir

    f32 = mybir.dt.float32
    bf16 = mybir.dt.bfloat16
    AF = mybir.ActivationFunctionType
    OP = mybir.AluOpType

    nc = bacc.Bacc(
        "TRN2", target_bir_lowering=False, debug=False, enable_asserts=False
    )

    # ---------------- DRAM I/O ----------------
    ht_d = nc.dram_tensor("ht", [IN, T, BS], bf16, kind="ExternalInput")
    hf_d = nc.dram_tensor("hf", [128, 32, HFW], bf16, kind="ExternalInput")
    wi2ht_d = nc.dram_tensor("wi2ht", [IN, HID], bf16, kind="ExternalInput")
    wh2ht_d = nc.dram_tensor("wh2ht", [HID, HID], bf16, kind="ExternalInput")
    bh2h_d = nc.dram_tensor("bh2h", [128, 4], f32, kind="ExternalInput")
    ws_d = nc.dram_tensor("ws", [128, 4], bf16, kind="ExternalInput")
    wcomb_d = nc.dram_tensor("wcomb", [IN + HID, G4], bf16, kind="ExternalInput")
    wio_d = nc.dram_tensor("wio", [97, G4], bf16, kind="ExternalInput")
    oh_d = nc.dram_tensor("oh", [97, S, BS], bf16, kind="ExternalInput")
    wgent_d = nc.dram_tensor("wgent", [HID, NCLS], bf16, kind="ExternalInput")
    bgen_d = nc.dram_tensor("bgen", [BS, NCLS], f32, kind="ExternalInput")
    probs_d = nc.dram_tensor("probs", [BS, S, NCLS], f32, kind="ExternalOutput")

    with tile.TileContext(nc, trace_sim=False) as tc, ExitStack() as ctx:
        sing = ctx.enter_context(tc.tile_pool(name="sing", bufs=1))

        def stile(shape, dt, tag):
            return sing.tile(shape, dt, tag=tag, name=tag)

        hflat = stile([128, 32, HFW], bf16, "hflat")
        hpT = stile([128, 4, T * BS], bf16, "hpT")
        wcomb = stile([128, 8, G4], bf16, "wcomb")
        wio = stile([97, G4], bf16, "wio")
        wh2ht = stile([128, 4, HID], bf16, "wh2ht")
        wgent = stile([128, 4, NCLS], bf16, "wgent")
        bh2h = stile([128, 4], f32, "bh2h")
        ws = stile([128, 4], bf16, "ws")
        oh = stile([97, S, BS], bf16, "oh")
        bgen = stile([BS, NCLS], f32, "bgen")
        id64f = stile([64, 64], f32, "id64f")
        id64b = stile([64, 64], bf16, "id64b")
        # step-state tiles, double-buffered by step parity
        hT = [stile([128, 4, BS], bf16, f"hT{i}") for i in range(2)]
        cst = [stile([BS, HID], f32, f"c{i}") for i in range(2)]
        bd = [stile([128, 32, BS], bf16, f"bd{i}") for i in range(2)]

        # ---------------- constant / weight loads ----------------
        nc.sync.dma_start(hflat[:], hf_d.ap())
        nc.sync.dma_start(
            wcomb[:], wcomb_d.ap().rearrange("(kb p) g -> p kb g", p=128)
        )
        nc.sync.dma_start(wio[:], wio_d.ap())
        nc.sync.dma_start(
            wh2ht[:], wh2ht_d.ap().rearrange("(kb p) h -> p kb h", p=128)
        )
        nc.sync.dma_start(
            wgent[:], wgent_d.ap().rearrange("(kb p) c -> p kb c", p=128)
        )
        nc.sync.dma_start(bh2h[:], bh2h_d.ap())
        nc.sync.dma_start(ws[:], ws_d.ap())
        nc.sync.dma_start(oh[:], oh_d.ap())
        nc.sync.dma_start(bgen[:], bgen_d.ap())
        masks.make_identity(nc, id64f[:])
        masks.make_identity(nc, id64b[:])
        # initial state: h = 0, c = 0; BD zeros (nonzero slots overwritten each step)
        nc.gpsimd.memset(hT[0][:], 0.0)
        nc.gpsimd.memset(cst[0][:], 0.0)
        nc.vector.memset(bd[0][:], 0.0)
        nc.vector.memset(bd[1][:], 0.0)

        # ---------------- precompute H_proj^T = W_i2h @ H^T ----------------
        with (
            tc.tile_pool(name="pre", bufs=1) as pre,
            tc.tile_pool(name="pp", bufs=3, space="PSUM") as pp,
        ):
            htt = pre.tile([128, 4, T * BS], bf16, tag="htt", name="htt")
            wi2ht = pre.tile([128, 4, HID], bf16, tag="wi2ht", name="wi2ht")
            nc.sync.dma_start(
                htt[:], ht_d.ap().rearrange("(kb p) t b -> p kb (t b)", p=128)
            )
            nc.sync.dma_start(
                wi2ht[:], wi2ht_d.ap().rearrange("(kb p) h -> p kb h", p=128)
            )
            for hb in range(4):
                for ns in range(8):
                    ps = pp.tile([128, 512], f32, tag="pp", name="pp")
                    for kb in range(4):
                        nc.tensor.matmul(
                            ps[:],
                            wi2ht[:, kb, hb * 128 : (hb + 1) * 128],
                            htt[:, kb, ns * 512 : (ns + 1) * 512],
                            start=(kb == 0),
                            stop=(kb == 3),
                        )
                    dst = hpT[:, hb, ns * 512 : (ns + 1) * 512]
                    if (hb * 8 + ns) % 2 == 0:
                        nc.vector.tensor_copy(dst, ps[:])
                    else:
                        nc.scalar.copy(dst, ps[:])

        # ---------------- step-loop pools ----------------
        att = ctx.enter_context(tc.tile_pool(name="attps", bufs=1, space="PSUM"))
        bigp = ctx.enter_context(tc.tile_pool(name="bigps", bufs=1, space="PSUM"))
        smallp = ctx.enter_context(tc.tile_pool(name="smps", bufs=2, space="PSUM"))
        sb = ctx.enter_context(tc.tile_pool(name="sbp", bufs=2))
        argp = ctx.enter_context(tc.tile_pool(name="argp", bufs=2))
        tanp = ctx.enter_context(tc.tile_pool(name="tanp", bufs=2))
        pw = ctx.enter_context(tc.tile_pool(name="pwp", bufs=8))

        for s in range(S):
            hT_prev = hT[s % 2]
            hT_next = hT[(s + 1) % 2]
            c_prev = cst[s % 2]
            c_next = cst[(s + 1) % 2]
            bds = bd[s % 2]

            # --- A: pT = W_h2h @ h^T + b_h2h  (transposed layout) ---
            pps = att.tile([128, 4, BS], f32, tag="att", name="pps")
            for hb in range(4):
                for kb in range(4):
                    nc.tensor.matmul(
                        pps[:, hb, :],
                        wh2ht[:, kb, hb * 128 : (hb + 1) * 128],
                        hT_prev[:, kb, :],
                        start=(kb == 0),
                        stop=(kb == 3),
                    )
            pT = sb.tile([128, 4, BS], bf16, tag="pT", name="pT")
            for hb in range(4):
                nc.vector.tensor_scalar(
                    pT[:, hb, :], pps[:, hb, :], bh2h[:, hb : hb + 1], None, OP.add
                )

            # --- B: arg = Hp^T + pT (bcast over t), tanh ---
            tts = []
            for hb in range(4):
                arg = argp.tile([128, T, BS], bf16, tag="arg", name="arg")
                nc.vector.tensor_tensor(
                    arg[:],
                    hpT[:, hb, :].rearrange("p (t b) -> p t b", t=T),
                    pT[:, hb, :].unsqueeze(1).broadcast_to((128, T, BS)),
                    OP.add,
                )
                th = tanp.tile([128, T, BS], bf16, tag="th", name="th")
                nc.scalar.activation(th[:], arg[:], AF.Tanh)
                tts.append(th)

            # --- C: e = w . tanh  -> e8 psum at 4 col-groups ---
            # hb-outer so each tanh tile is fully consumed right after ACT
            # produces it (tanh of block k overlaps score of block k-1 on PE).
            e8 = att.tile([128, 2, 512], f32, tag="att", name="e8")
            for hb in range(4):
                th_flat = tts[hb][:].rearrange("p t b -> p (t b)")
                for ns in range(8):
                    g, q = divmod(ns, 2)
                    nc.tensor.matmul(
                        e8[32 * g : 32 * g + 1, q, :],
                        ws[:, hb : hb + 1],
                        th_flat[:, ns * 512 : (ns + 1) * 512],
                        start=(hb == 0),
                        stop=(hb == 3),
                        tile_position=(0, 32 * g),
                    )

            # --- D: e -> SBUF (4 one-lane copies), reshape via 8 small DMAs,
            #        exp at full width, scatter into block-diag alpha ---
            e_sb = sb.tile([128, 2, 512], f32, tag="aE", name="e_sb")
            for g in range(4):
                src = e8[32 * g : 32 * g + 1, :, :]
                dst = e_sb[32 * g : 32 * g + 1, :, :]
                if g < 2:
                    nc.vector.tensor_copy(dst, src)
                else:
                    nc.scalar.copy(dst, src)
            eT = sb.tile([128, BS], f32, tag="a128", name="eT")
            for g in range(4):
                src = e_sb[32 * g : 32 * g + 1, :, :].rearrange(
                    "p q n -> p (q n)"
                )
                for half in range(2):
                    base = 64 * half + 16 * g
                    nc.sync.dma_start(eT[base : base + 16, :], src)
            alphaT = sb.tile([128, BS], bf16, tag="alphaT", name="alphaT")
            nc.scalar.activation(alphaT[:], eT[:], AF.Exp)
            bd_flat = bds[:].rearrange("p j b -> p (j b)")
            for half in range(2):
                nc.vector.tensor_copy(
                    bd_flat[
                        half * 64 : (half + 1) * 64, half : half + 66 * 31 + 1 : 66
                    ],
                    alphaT[half * 64 : (half + 1) * 64, half::2],
                )

            # --- E: context (+ softmax denom via ones column) ---
            ctxps = bigp.tile([64, HFW], f32, tag="big", name="ctxps")
            for j in range(32):
                nc.tensor.matmul(
                    ctxps[:, 0:512],
                    bds[:, j, :],
                    hflat[:, j, 0:512],
                    start=(j == 0),
                    stop=(j == 31),
                )
            for j in range(32):
                nc.tensor.matmul(
                    ctxps[:, 512:516],
                    bds[:, j, :],
                    hflat[:, j, 512:516],
                    start=(j == 0),
                    stop=(j == 31),
                )
            rS = sb.tile([64, 1], f32, tag="rS", name="rS")
            nc.vector.reciprocal(rS[:], ctxps[:, 512:513])
            ctx_sb = sb.tile([64, IN], bf16, tag="ctxsb", name="ctxsb")
            nc.vector.tensor_scalar(
                ctx_sb[:], ctxps[:, 0:512], rS[:], None, OP.mult
            )

            # --- F: transpose ctx -> ctxT (PE, bf16) ---
            ctxT = sb.tile([128, 4, 64], bf16, tag="ctxT", name="ctxT")
            for kb in range(4):
                tp = smallp.tile([128, 64], bf16, tag="sm", name="tpb")
                nc.tensor.transpose(
                    tp[:], ctx_sb[:, kb * 128 : (kb + 1) * 128], id64b[:]
                )
                nc.vector.tensor_copy(ctxT[:, kb, :], tp[:])

            # --- G: LSTM gates = Wcomb^T @ [ctx; h] + W_io^T @ onehot (+biases) ---
            gates = bigp.tile([64, G4], f32, tag="big", name="gates")
            for nn in range(4):
                gslice = gates[:, nn * 512 : (nn + 1) * 512]
                for kb in range(8):
                    lhsT = ctxT[:, kb, :] if kb < 4 else hT_prev[:, kb - 4, :]
                    nc.tensor.matmul(
                        gslice,
                        lhsT,
                        wcomb[:, kb, nn * 512 : (nn + 1) * 512],
                        start=(kb == 0),
                        stop=False,
                    )
                nc.tensor.matmul(
                    gslice,
                    oh[:, s, :],
                    wio[:, nn * 512 : (nn + 1) * 512],
                    start=False,
                    stop=True,
                )

            # --- H: pointwise LSTM (sigmoid via tanh) ---
            def pwt(tag="pw"):
                return pw.tile([BS, HID], f32, tag=tag, name="pw")

            ti, tf, tg, to = pwt(), pwt(), pwt(), pwt()
            nc.scalar.activation(ti[:], gates[:, 0:512], AF.Tanh, scale=0.5)
            nc.scalar.activation(tf[:], gates[:, 512:1024], AF.Tanh, scale=0.5)
            nc.scalar.activation(tg[:], gates[:, 1024:1536], AF.Tanh)
            nc.scalar.activation(to[:], gates[:, 1536:2048], AF.Tanh, scale=0.5)
            ig, fg, og = pwt(), pwt(), pwt()
            nc.vector.tensor_scalar(ig[:], ti[:], 0.5, 0.5, OP.mult, OP.add)
            nc.vector.tensor_scalar(fg[:], tf[:], 0.5, 0.5, OP.mult, OP.add)
            nc.vector.tensor_scalar(og[:], to[:], 0.5, 0.5, OP.mult, OP.add)
            m1, m2 = pwt(), pwt()
            nc.vector.tensor_mul(m1[:], fg[:], c_prev[:])
            nc.vector.tensor_mul(m2[:], ig[:], tg[:])
            nc.vector.tensor_add(c_next[:], m1[:], m2[:])
            tc_t = pwt()
            nc.scalar.activation(tc_t[:], c_next[:], AF.Tanh)
            h_new = pwt()
            nc.vector.tensor_mul(h_new[:], og[:], tc_t[:])

            # --- I: transpose h -> hT (PE, f32 -> bf16) ---
            for kb in range(4):
                tp = smallp.tile([128, 64], f32, tag="sm", name="tpf")
                nc.tensor.transpose(
                    tp[:], h_new[:, kb * 128 : (kb + 1) * 128], id64f[:]
                )
                nc.vector.tensor_copy(hT_next[:, kb, :], tp[:])

            # --- J: generator: probs_s = h @ W_gen^T + b_gen ---
            gps = smallp.tile([64, NCLS], f32, tag="sm", name="gps")
            for kb in range(4):
                nc.tensor.matmul(
                    gps[:],
                    hT_next[:, kb, :],
                    wgent[:, kb, :],
                    start=(kb == 0),
                    stop=(kb == 3),
                )
            pr = sb.tile([64, NCLS], f32, tag="pr", name="pr")
            nc.vector.tensor_add(pr[:], gps[:], bgen[:])
            nc.sync.dma_start(probs_d.ap()[:, s, :], pr[:])

    nc.compile()
    return nc


def _get_nc():
    if "nc" not in _CACHE:
        _CACHE["nc"] = _build_nc()
    return _CACHE["nc"]


def _prep_core_inputs(inputs):
    """Host-side data prep: shard, transpose, cast. Returns list of in_maps."""
    bH = np.asarray(inputs["batch_H"], np.float32)
    text = np.asarray(inputs["text"])
    W_i2h = np.asarray(inputs["W_i2h"], np.float32)
    W_h2h = np.asarray(inputs["W_h2h"], np.float32)
    b_h2h = np.asarray(inputs["b_h2h"], np.float32)
    w_score = np.asarray(inputs["w_score"], np.float32)[0]
    W_ih = np.asarray(inputs["W_ih"], np.float32)
    b_ih = np.asarray(inputs["b_ih"], np.float32)
    W_hh = np.asarray(inputs["W_hh"], np.float32)
    b_hh = np.asarray(inputs["b_hh"], np.float32)
    W_gen = np.asarray(inputs["W_gen"], np.float32)
    b_gen = np.asarray(inputs["b_gen"], np.float32)

    wi2ht = np.ascontiguousarray(W_i2h.T).astype(BF)
    wh2ht = np.ascontiguousarray(W_h2h.T).astype(BF)
    bh2h = np.ascontiguousarray(b_h2h.reshape(4, 128).T).astype(np.float32)
    ws = np.ascontiguousarray(w_score.reshape(4, 128).T).astype(BF)
    wcomb = np.ascontiguousarray(
        np.concatenate([W_ih[:, :IN], W_hh], axis=1).T
    ).astype(BF)
    wio = np.zeros((97, G4), BF)
    wio[:96] = W_ih[:, IN:].T.astype(BF)
    wio[96] = (b_ih + b_hh).astype(BF)
    wgent = np.ascontiguousarray(W_gen.T).astype(BF)
    bgen = np.ascontiguousarray(np.broadcast_to(b_gen, (BS, NCLS))).astype(
        np.float32
    )

    in_maps = []
    bidx = np.arange(BS)
    for c in range(NCORES):
        sh = slice(c * BS, (c + 1) * BS)
        Hs = bH[sh]                                   # [64, 64, 512]
        ht = np.ascontiguousarray(Hs.transpose(2, 1, 0)).astype(BF)
        hf = np.zeros((128, 32, HFW), BF)
        hf[:, :, :IN] = (
            Hs.reshape(32, 2, T, IN).transpose(1, 2, 0, 3).reshape(128, 32, IN)
        ).astype(BF)
        hf[:, :, IN] = 1.0
        ts = np.asarray(text[sh])                     # [64, 26]
        oh97 = np.zeros((97, S, BS), BF)
        for s in range(S):
            oh97[ts[:, s], s, bidx] = 1.0
        oh97[96] = 1.0
        in_maps.append(
            {
                "ht": ht,
                "hf": hf,
                "wi2ht": wi2ht,
                "wh2ht": wh2ht,
                "bh2h": bh2h,
                "ws": ws,
                "wcomb": wcomb,
                "wio": wio,
                "oh": oh97,
                "wgent": wgent,
                "bgen": bgen,
            }
        )
    return in_maps


def kernel(**inputs):
    from concourse.bass_utils import run_bass_kernel_spmd

    nc = _get_nc()
    in_maps = _prep_core_inputs(inputs)
    res = run_bass_kernel_spmd(nc, in_maps, core_ids=list(range(NCORES)))
    out = np.concatenate([r["probs"] for r in res.results], axis=0)
    return out.astype(np.float32)


if __name__ == "__main__":
    # smoke test with random inputs of the right shapes
    rng = np.random.default_rng(0)
    fake = {
        "batch_H": rng.standard_normal((B, T, IN), dtype=np.float32),
        "text": rng.integers(0, NCLS, (B, S)),
        "W_i2h": rng.standard_normal((HID, IN), dtype=np.float32) * 0.04,
        "W_h2h": rng.standard_normal((HID, HID), dtype=np.float32) * 0.04,
        "b_h2h": rng.standard_normal(HID, dtype=np.float32) * 0.04,
        "w_score": rng.standard_normal((1, HID), dtype=np.float32) * 0.04,
        "W_ih": rng.standard_normal((G4, IN + NCLS), dtype=np.float32) * 0.04,
        "b_ih": rng.standard_normal(G4, dtype=np.float32) * 0.04,
        "W_hh": rng.standard_normal((G4, HID), dtype=np.float32) * 0.04,
        "b_hh": rng.standard_normal(G4, dtype=np.float32) * 0.04,
        "W_gen": rng.standard_normal((NCLS, HID), dtype=np.float32) * 0.04,
        "b_gen": rng.standard_normal(NCLS, dtype=np.float32) * 0.04,
    }
    out = kernel(**fake)
    print("kernel output:", out.shape, out.dtype, float(np.abs(out).max()))


# revision 13
# speedup vs baseline: 51.5024x; 51.5024x over previous
"""Trainium2 Bass kernel: attention-LSTM decoder (teacher-forced, 26 steps).

Full inputs -> data-parallel over batch across 8 NeuronCores -> full output.

Per-core layout strategy (B_shard = 64):
  - H_proj^T   [h(4x128), (t,b)]  bf16, precomputed on PE once
  - per step:  pT = W_h2h @ h^T (PE)  ->  arg = Hp^T + pT (DVE bcast add, bf16)
               -> tanh (ACT) -> score dot vs w (PE, 4 PSUM col-groups)
               -> exp (ACT, 4 lanes) -> DMA reshape -> block-diag alpha
               -> context matmul w/ ones-column for softmax denom (PE)
               -> ctx scale 1/S (DVE) -> transpose (PE) -> LSTM gates (PE)
               -> pointwise LSTM with sigmoid(x)=0.5*tanh(x/2)+0.5 (ACT+DVE)
               -> h^T transpose (PE) -> generator matmul (PE) -> DMA out
  All transcendentals use the exp_and_others ACT table set (tanh+exp): a
  single table load for the whole kernel.
"""

import numpy as np
import ml_dtypes

B, T, IN = 512, 64, 512
HID = 512
NCLS = 96
S = 26
NCORES = 8
BS = B // NCORES           # 64 batch rows per core
G4 = 4 * HID               # 2048 LSTM gate width
HFW = IN + 4               # Hflat chunk width: 512 H cols + ones col + 3 pad
BF = ml_dtypes.bfloat16
REPEAT = 1                 # timing knob: run the step loop this many times

_CACHE = {}


def _build_nc():
    from contextlib import ExitStack

    import concourse.masks as masks
    import concourse.tile as tile
    from concourse import bacc, mybir

    f32 = mybir.dt.float32
    bf16 = mybir.dt.bfloat16
    AF = mybir.ActivationFunctionType
    OP = mybir.AluOpType

    nc = bacc.Bacc(
        "TRN2", target_bir_lowering=False, debug=False, enable_asserts=False
    )

    # ---------------- DRAM I/O ----------------
    ht_d = nc.dram_tensor("ht", [IN, T, BS], bf16, kind="ExternalInput")
    hf_d = nc.dram_tensor("hf", [128, 32, HFW], bf16, kind="ExternalInput")
    wi2ht_d = nc.dram_tensor("wi2ht", [IN, HID], bf16, kind="ExternalInput")
    wh2ht_d = nc.dram_tensor("wh2ht", [HID, HID], bf16, kind="ExternalInput")
    bh2h_d = nc.dram_tensor("bh2h", [128, 4], f32, kind="ExternalInput")
    ws_d = nc.dram_tensor("ws", [128, 4], bf16, kind="ExternalInput")
    wcomb_d = nc.dram_tensor("wcomb", [IN + HID, G4], bf16, kind="ExternalInput")
    wio_d = nc.dram_tensor("wio", [97, G4], bf16, kind="ExternalInput")
    oh_d = nc.dram_tensor("oh", [97, S, BS], bf16, kind="ExternalInput")
    wgent_d = nc.dram_tensor("wgent", [HID, NCLS], bf16, kind="ExternalInput")
    bgen_d = nc.dram_tensor("bgen", [BS, NCLS], f32, kind="ExternalInput")
    probs_d = nc.dram_tensor("probs", [BS, S, NCLS], f32, kind="ExternalOutput")

    with tile.TileContext(nc, trace_sim=False) as tc, ExitStack() as ctx:
        sing = ctx.enter_context(tc.tile_pool(name="sing", bufs=1))

        def stile(shape, dt, tag):
            return sing.tile(shape, dt, tag=tag, name=tag)

        hflat = stile([128, 32, HFW], bf16, "hflat")
        hpT = stile([128, 4, T * BS], bf16, "hpT")
        wcomb = stile([128, 8, G4], bf16, "wcomb")
        wio = stile([97, G4], bf16, "wio")
        wh2ht = stile([128, 4, HID], bf16, "wh2ht")
        wgent = stile([128, 4, NCLS], bf16, "wgent")
        bh2h = stile([128, 4], f32, "bh2h")
        ws = stile([128, 4], bf16, "ws")
        oh = stile([97, S, BS], bf16, "oh")
        bgen = stile([BS, NCLS], f32, "bgen")
        id64f = stile([64, 64], f32, "id64f")
        id64b = stile([64, 64], bf16, "id64b")
        # step-state tiles, double-buffered by step parity
        hT = [stile([128, 4, BS], bf16, f"hT{i}") for i in range(2)]
        cst = [stile([BS, HID], f32, f"c{i}") for i in range(2)]
        bd = [stile([128, 32, BS], bf16, f"bd{i}") for i in range(2)]

        # ---------------- constant / weight loads ----------------
        nc.sync.dma_start(hflat[:], hf_d.ap())
        nc.sync.dma_start(
            wcomb[:], wcomb_d.ap().rearrange("(kb p) g -> p kb g", p=128)
        )
        nc.sync.dma_start(wio[:], wio_d.ap())
        nc.sync.dma_start(
            wh2ht[:], wh2ht_d.ap().rearrange("(kb p) h -> p kb h", p=128)
        )
        nc.sync.dma_start(
            wgent[:], wgent_d.ap().rearrange("(kb p) c -> p kb c", p=128)
        )
        nc.sync.dma_start(bh2h[:], bh2h_d.ap())
        nc.sync.dma_start(ws[:], ws_d.ap())
        nc.sync.dma_start(oh[:], oh_d.ap())
        nc.sync.dma_start(bgen[:], bgen_d.ap())
        masks.make_identity(nc, id64f[:])
        masks.make_identity(nc, id64b[:])
        # initial state: h = 0, c = 0; BD zeros (nonzero slots overwritten each step)
        nc.gpsimd.memset(hT[0][:], 0.0)
        nc.gpsimd.memset(cst[0][:], 0.0)
        nc.vector.memset(bd[0][:], 0.0)
        nc.vector.memset(bd[1][:], 0.0)

        # ---------------- precompute H_proj^T = W_i2h @ H^T ----------------
        with (
            tc.tile_pool(name="pre", bufs=1) as pre,
            tc.tile_pool(name="pp", bufs=3, space="PSUM") as pp,
        ):
            htt = pre.tile([128, 4, T * BS], bf16, tag="htt", name="htt")
            wi2ht = pre.tile([128, 4, HID], bf16, tag="wi2ht", name="wi2ht")
            nc.sync.dma_start(
                htt[:], ht_d.ap().rearrange("(kb p) t b -> p kb (t b)", p=128)
            )
            nc.sync.dma_start(
                wi2ht[:], wi2ht_d.ap().rearrange("(kb p) h -> p kb h", p=128)
            )
            for hb in range(4):
                for ns in range(8):
                    ps = pp.tile([128, 512], f32, tag="pp", name="pp")
                    for kb in range(4):
                        nc.tensor.matmul(
                            ps[:],
                            wi2ht[:, kb, hb * 128 : (hb + 1) * 128],
                            htt[:, kb, ns * 512 : (ns + 1) * 512],
                            start=(kb == 0),
                            stop=(kb == 3),
                        )
                    dst = hpT[:, hb, ns * 512 : (ns + 1) * 512]
                    if (hb * 8 + ns) % 2 == 0:
                        nc.vector.tensor_copy(dst, ps[:])
                    else:
                        nc.scalar.copy(dst, ps[:])

        # ---------------- step-loop pools ----------------
        att = ctx.enter_context(tc.tile_pool(name="attps", bufs=1, space="PSUM"))
        bigp = ctx.enter_context(tc.tile_pool(name="bigps", bufs=1, space="PSUM"))
        smallp = ctx.enter_context(tc.tile_pool(name="smps", bufs=2, space="PSUM"))
        sb = ctx.enter_context(tc.tile_pool(name="sbp", bufs=2))
        argp = ctx.enter_context(tc.tile_pool(name="argp", bufs=2))
        tanp = ctx.enter_context(tc.tile_pool(name="tanp", bufs=2))
        pw = ctx.enter_context(tc.tile_pool(name="pwp", bufs=8))

        for step in range(REPEAT * S):
            s = step % S
            hT_prev = hT[step % 2]
            hT_next = hT[(step + 1) % 2]
            c_prev = cst[step % 2]
            c_next = cst[(step + 1) % 2]
            bds = bd[step % 2]

            # --- A: pT = W_h2h @ h^T + b_h2h  (transposed layout) ---
            pps = att.tile([128, 4, BS], f32, tag="att", name="pps")
            for hb in range(4):
                for kb in range(4):
                    nc.tensor.matmul(
                        pps[:, hb, :],
                        wh2ht[:, kb, hb * 128 : (hb + 1) * 128],
                        hT_prev[:, kb, :],
                        start=(kb == 0),
                        stop=(kb == 3),
                    )
            pT = sb.tile([128, 4, BS], bf16, tag="pT", name="pT")
            for hb in range(4):
                nc.vector.tensor_scalar(
                    pT[:, hb, :], pps[:, hb, :], bh2h[:, hb : hb + 1], None, OP.add
                )

            # --- B: arg = Hp^T + pT (bcast over t), tanh ---
            tts = []
            for hb in range(4):
                arg = argp.tile([128, T, BS], bf16, tag="arg", name="arg")
                nc.vector.tensor_tensor(
                    arg[:],
                    hpT[:, hb, :].rearrange("p (t b) -> p t b", t=T),
                    pT[:, hb, :].unsqueeze(1).broadcast_to((128, T, BS)),
                    OP.add,
                )
                th = tanp.tile([128, T, BS], bf16, tag="th", name="th")
                nc.scalar.activation(th[:], arg[:], AF.Tanh)
                tts.append(th)

            # --- C: e = w . tanh  -> e8 psum at 4 col-groups ---
            # hb-outer so each tanh tile is fully consumed right after ACT
            # produces it (tanh of block k overlaps score of block k-1 on PE).
            e8 = att.tile([128, 2, 512], f32, tag="att", name="e8")
            for hb in range(4):
                th_flat = tts[hb][:].rearrange("p t b -> p (t b)")
                for ns in range(8):
                    g, q = divmod(ns, 2)
                    nc.tensor.matmul(
                        e8[32 * g : 32 * g + 1, q, :],
                        ws[:, hb : hb + 1],
                        th_flat[:, ns * 512 : (ns + 1) * 512],
                        start=(hb == 0),
                        stop=(hb == 3),
                        tile_position=(0, 32 * g),
                    )

            # --- D: e -> SBUF (4 one-lane copies), reshape via 8 small DMAs,
            #        exp at full width, scatter into block-diag alpha ---
            e_sb = sb.tile([128, 2, 512], f32, tag="aE", name="e_sb")
            for g in range(4):
                src = e8[32 * g : 32 * g + 1, :, :]
                dst = e_sb[32 * g : 32 * g + 1, :, :]
                if g < 2:
                    nc.vector.tensor_copy(dst, src)
                else:
                    nc.scalar.copy(dst, src)
            eT = sb.tile([128, BS], f32, tag="a128", name="eT")
            for g in range(4):
                src = e_sb[32 * g : 32 * g + 1, :, :].rearrange(
                    "p q n -> p (q n)"
                )
                for half in range(2):
                    base = 64 * half + 16 * g
                    nc.sync.dma_start(eT[base : base + 16, :], src)
            alphaT = sb.tile([128, BS], bf16, tag="alphaT", name="alphaT")
            nc.scalar.activation(alphaT[:], eT[:], AF.Exp)
            bd_flat = bds[:].rearrange("p j b -> p (j b)")
            for half in range(2):
                nc.vector.tensor_copy(
                    bd_flat[
                        half * 64 : (half + 1) * 64, half : half + 66 * 31 + 1 : 66
                    ],
                    alphaT[half * 64 : (half + 1) * 64, half::2],
                )

            # --- E: context (+ softmax denom via ones column) ---
            ctxps = bigp.tile([64, HFW], f32, tag="big", name="ctxps")
            for j in range(32):
                nc.tensor.matmul(
                    ctxps[:, 0:512],
                    bds[:, j, :],
                    hflat[:, j, 0:512],
                    start=(j == 0),
                    stop=(j == 31),
                )
            for j in range(32):
                nc.tensor.matmul(
                    ctxps[:, 512:516],
                    bds[:, j, :],
                    hflat[:, j, 512:516],
                    start=(j == 0),
                    stop=(j == 31),
                )
            rS = sb.tile([64, 1], f32, tag="rS", name="rS")
            nc.vector.reciprocal(rS[:], ctxps[:, 512:513])
            ctx_sb = sb.tile([64, IN], bf16, tag="ctxsb", name="ctxsb")
            nc.vector.tensor_scalar(
                ctx_sb[:], ctxps[:, 0:512], rS[:], None, OP.mult
            )

            # --- F: transpose ctx -> ctxT (PE, bf16) ---
            ctxT = sb.tile([128, 4, 64], bf16, tag="ctxT", name="ctxT")
            for kb in range(4):
                tp = smallp.tile([128, 64], bf16, tag="sm", name="tpb")
                nc.tensor.transpose(
                    tp[:], ctx_sb[:, kb * 128 : (kb + 1) * 128], id64b[:]
                )
                nc.vector.tensor_copy(ctxT[:, kb, :], tp[:])

            # --- G: LSTM gates = Wcomb^T @ [ctx; h] + W_io^T @ onehot (+biases) ---
            gates = bigp.tile([64, G4], f32, tag="big", name="gates")
            for nn in range(4):
                gslice = gates[:, nn * 512 : (nn + 1) * 512]
                for kb in range(8):
                    lhsT = ctxT[:, kb, :] if kb < 4 else hT_prev[:, kb - 4, :]
                    nc.tensor.matmul(
                        gslice,
                        lhsT,
                        wcomb[:, kb, nn * 512 : (nn + 1) * 512],
                        start=(kb == 0),
                        stop=False,
                    )
                nc.tensor.matmul(
                    gslice,
                    oh[:, s, :],
                    wio[:, nn * 512 : (nn + 1) * 512],
                    start=False,
                    stop=True,
                )

            # --- H: pointwise LSTM (sigmoid via tanh) ---
            def pwt(tag="pw"):
                return pw.tile([BS, HID], f32, tag=tag, name="pw")

            ti, tf, tg, to = pwt(), pwt(), pwt(), pwt()
            nc.scalar.activation(ti[:], gates[:, 0:512], AF.Tanh, scale=0.5)
            nc.scalar.activation(tf[:], gates[:, 512:1024], AF.Tanh, scale=0.5)
            nc.scalar.activation(tg[:], gates[:, 1024:1536], AF.Tanh)
            nc.scalar.activation(to[:], gates[:, 1536:2048], AF.Tanh, scale=0.5)
            ig, fg, og = pwt(), pwt(), pwt()
            nc.vector.tensor_scalar(ig[:], ti[:], 0.5, 0.5, OP.mult, OP.add)
            nc.vector.tensor_scalar(fg[:], tf[:], 0.5, 0.5, OP.mult, OP.add)
            nc.vector.tensor_scalar(og[:], to[:], 0.5, 0.5, OP.mult, OP.add)
            m1, m2 = pwt(), pwt()
            nc.vector.tensor_mul(m1[:], fg[:], c_prev[:])
            nc.vector.tensor_mul(m2[:], ig[:], tg[:])
            nc.vector.tensor_add(c_next[:], m1[:], m2[:])
            tc_t = pwt()
            nc.scalar.activation(tc_t[:], c_next[:], AF.Tanh)
            h_new = pwt()
            nc.vector.tensor_mul(h_new[:], og[:], tc_t[:])

            # --- I: transpose h -> hT (PE, f32 -> bf16) ---
            for kb in range(4):
                tp = smallp.tile([128, 64], f32, tag="sm", name="tpf")
                nc.tensor.transpose(
                    tp[:], h_new[:, kb * 128 : (kb + 1) * 128], id64f[:]
                )
                nc.vector.tensor_copy(hT_next[:, kb, :], tp[:])

            # --- J: generator: probs_s = h @ W_gen^T + b_gen ---
            gps = smallp.tile([64, NCLS], f32, tag="sm", name="gps")
            for kb in range(4):
                nc.tensor.matmul(
                    gps[:],
                    hT_next[:, kb, :],
                    wgent[:, kb, :],
                    start=(kb == 0),
                    stop=(kb == 3),
                )
            pr = sb.tile([64, NCLS], f32, tag="pr", name="pr")
            nc.vector.tensor_add(pr[:], gps[:], bgen[:])
            nc.sync.dma_start(probs_d.ap()[:, s, :], pr[:])

    nc.compile()
    return nc


def _get_nc():
    if "nc" not in _CACHE:
        _CACHE["nc"] = _build_nc()
    return _CACHE["nc"]


def _prep_core_inputs(inputs):
    """Host-side data prep: shard, transpose, cast. Returns list of in_maps."""
    bH = np.asarray(inputs["batch_H"], np.float32)
    text = np.asarray(inputs["text"])
    W_i2h = np.asarray(inputs["W_i2h"], np.float32)
    W_h2h = np.asarray(inputs["W_h2h"], np.float32)
    b_h2h = np.asarray(inputs["b_h2h"], np.float32)
    w_score = np.asarray(inputs["w_score"], np.float32)[0]
    W_ih = np.asarray(inputs["W_ih"], np.float32)
    b_ih = np.asarray(inputs["b_ih"], np.float32)
    W_hh = np.asarray(inputs["W_hh"], np.float32)
    b_hh = np.asarray(inputs["b_hh"], np.float32)
    W_gen = np.asarray(inputs["W_gen"], np.float32)
    b_gen = np.asarray(inputs["b_gen"], np.float32)

    wi2ht = np.ascontiguousarray(W_i2h.T).astype(BF)
    wh2ht = np.ascontiguousarray(W_h2h.T).astype(BF)
    bh2h = np.ascontiguousarray(b_h2h.reshape(4, 128).T).astype(np.float32)
    ws = np.ascontiguousarray(w_score.reshape(4, 128).T).astype(BF)
    wcomb = np.ascontiguousarray(
        np.concatenate([W_ih[:, :IN], W_hh], axis=1).T
    ).astype(BF)
    wio = np.zeros((97, G4), BF)
    wio[:96] = W_ih[:, IN:].T.astype(BF)
    wio[96] = (b_ih + b_hh).astype(BF)
    wgent = np.ascontiguousarray(W_gen.T).astype(BF)
    bgen = np.ascontiguousarray(np.broadcast_to(b_gen, (BS, NCLS))).astype(
        np.float32
    )

    in_maps = []
    bidx = np.arange(BS)
    for c in range(NCORES):
        sh = slice(c * BS, (c + 1) * BS)
        Hs = bH[sh]                                   # [64, 64, 512]
        ht = np.ascontiguousarray(Hs.transpose(2, 1, 0)).astype(BF)
        hf = np.zeros((128, 32, HFW), BF)
        hf[:, :, :IN] = (
            Hs.reshape(32, 2, T, IN).transpose(1, 2, 0, 3).reshape(128, 32, IN)
        ).astype(BF)
        hf[:, :, IN] = 1.0
        ts = np.asarray(text[sh])                     # [64, 26]
        oh97 = np.zeros((97, S, BS), BF)
        for s in range(S):
            oh97[ts[:, s], s, bidx] = 1.0
        oh97[96] = 1.0
        in_maps.append(
            {
                "ht": ht,
                "hf": hf,
                "wi2ht": wi2ht,
                "wh2ht": wh2ht,
                "bh2h": bh2h,
                "ws": ws,
                "wcomb": wcomb,
                "wio": wio,
                "oh": oh97,
                "wgent": wgent,
                "bgen": bgen,
            }
        )
    return in_maps


def kernel(**inputs):
    from concourse.bass_utils import run_bass_kernel_spmd

    nc = _get_nc()
    in_maps = _prep_core_inputs(inputs)
    res = run_bass_kernel_spmd(nc, in_maps, core_ids=list(range(NCORES)))
    out = np.concatenate([r["probs"] for r in res.results], axis=0)
    return out.astype(np.float32)


if __name__ == "__main__":
    # smoke test with random inputs of the right shapes
    rng = np.random.default_rng(0)
    fake = {
        "batch_H": rng.standard_normal((B, T, IN), dtype=np.float32),
        "text": rng.integers(0, NCLS, (B, S)),
        "W_i2h": rng.standard_normal((HID, IN), dtype=np.float32) * 0.04,
        "W_h2h": rng.standard_normal((HID, HID), dtype=np.float32) * 0.04,
        "b_h2h": rng.standard_normal(HID, dtype=np.float32) * 0.04,
        "w_score": rng.standard_normal((1, HID), dtype=np.float32) * 0.04,
        "W_ih": rng.standard_normal((G4, IN + NCLS), dtype=np.float32) * 0.04,
        "b_ih": rng.standard_normal(G4, dtype=np.float32) * 0.04,
        "W_hh": rng.standard_normal((G4, HID), dtype=np.float32) * 0.04,
        "b_hh": rng.standard_normal(G4, dtype=np.float32) * 0.04,
        "W_gen": rng.standard_normal((NCLS, HID), dtype=np.float32) * 0.04,
        "b_gen": rng.standard_normal(NCLS, dtype=np.float32) * 0.04,
    }
    out = kernel(**fake)
    print("kernel output:", out.shape, out.dtype, float(np.abs(out).max()))
